# revision 29
# baseline (speedup 1.0000x reference)
"""Bottom-k cross-entropy loss on 8 Trainium2 NeuronCores.

Per-sample CE over [8192, 32000] logits, then mean of the 4096 smallest
losses.  Data-parallel: rows sharded across 8 cores; each core streams its
131MB shard once (memory-bound), computes local CE via one fused
exp+accumulate pass on the scalar engine, all-gathers the 8192 losses
(tiny), and every core redundantly runs an exact threshold-refinement
selection (2 rounds x 128 brackets, then a tie-corrected min/relu sum,
split across the vector and scalar engines) to produce the bottom-k mean.

Selection math: brackets are multiples of 2^-9 < 32, so all threshold
arithmetic is exact in f32.  With t >= v_(m) within one final bracket,
  mean_bottom_m = (sum_A min(x, t) - sum_B relu(t - x)) / m
(the t terms cancel exactly when m = N/2) is exact up to
(C(t)-m)*bracket_width/m ~ 1e-6 absolute.
"""

import numpy as np

N_CORES = 8
N_FULL, V_FULL = 8192, 32000
P = 128

# bracket steps: ranges 32, 0.25, 2^-9; all CE values lie in (0, 32]
S1, S2, S3 = 2.0**-2, 2.0**-9, 2.0**-16


def build_nc(n_cores, r, v, f):
    """Build the SPMD Bass program (identical on every core)."""
    from concourse import bass, bacc, mybir, tile

    assert r % P == 0 and v % f == 0
    rb_n = r // P
    nch = v // f
    ng = r * n_cores
    m = ng // 2
    f32 = mybir.dt.float32

    nc = bacc.Bacc()
    x = nc.declare_dram_parameter("x", [r, v], f32, isOutput=False)
    offs = nc.declare_dram_parameter("offs", [P, rb_n], mybir.dt.int32, isOutput=False)
    iota1 = nc.declare_dram_parameter("iota1", [P, 1], f32, isOutput=False)
    iota1n = nc.declare_dram_parameter("iota1n", [P, 1], f32, isOutput=False)
    iota2 = nc.declare_dram_parameter("iota2", [P, 1], f32, isOutput=False)
    out = nc.declare_dram_parameter("out", [1, 1], f32, isOutput=True)

    with tile.TileContext(nc) as tc:
        with (
            tc.tile_pool(name="dram", bufs=1, space="DRAM") as dpool,
            tc.tile_pool(name="consts", bufs=1) as cpool,
            tc.tile_pool(name="xs", bufs=5) as xpool,
            tc.tile_pool(name="es", bufs=2) as epool,
            tc.tile_pool(name="stats", bufs=2) as spool,
            tc.tile_pool(name="sel", bufs=1) as selpool,
            tc.tile_pool(name="psum", bufs=2, space="PSUM") as ppool,
        ):
            ce_local = dpool.tile([r, 1], f32, name="ce_local")
            ce_all = dpool.tile([ng, 1], f32, addr_space="Shared", name="ce_all")
            offs_sb = cpool.tile([P, rb_n], mybir.dt.int32)
            nc.gpsimd.dma_start(offs_sb[:], offs[:])
            io1 = cpool.tile([P, 1], f32)
            nc.gpsimd.dma_start(io1[:], iota1[:])
            io1n = cpool.tile([P, 1], f32)
            nc.gpsimd.dma_start(io1n[:], iota1n[:])
            io2 = cpool.tile([P, 1], f32)
            nc.gpsimd.dma_start(io2[:], iota2[:])

            # tiny dummy partition_broadcast: forces the gpsimd ucode library
            # load to happen here (gpsimd is idle during streaming) instead of
            # right before the real broadcast in the latency-critical tail
            dsrc = cpool.tile([1, 4], f32)
            nc.vector.memset(dsrc[:], 0.0)
            dout = cpool.tile([P, 4], f32)
            nc.gpsimd.partition_broadcast(dout[:], dsrc[:])

            # gather picked logits: x.flat[row*v + label] for each local row
            picked = cpool.tile([P, rb_n], f32)
            x_flat = x[:].rearrange("a b -> (a b) ()")
            for rbi in range(rb_n):
                nc.gpsimd.indirect_dma_start(
                    out=picked[:, rbi : rbi + 1],
                    out_offset=None,
                    in_=x_flat,
                    in_offset=bass.IndirectOffsetOnAxis(
                        ap=offs_sb[:, rbi : rbi + 1], axis=0
                    ),
                )

            # streaming pass: pure DMA + fused exp/accumulate, no mid-stream
            # epilogues (table switches and dependent stores would stall the
            # sync HWDGE ring and the ACT queue at row-block boundaries)
            part_all = spool.tile([P, rb_n * nch], f32)
            for rbi in range(rb_n):
                for c in range(nch):
                    xt = xpool.tile([P, f], f32, tag="xt")
                    # split loads across the sync HWDGE ring and the gpsimd
                    # SWDGE ring; the gpsimd sequencer runs no compute during
                    # streaming, so its trigger waits cannot interlock with
                    # the exp stream (unlike the ACT-engine HWDGE ring)
                    dma_eng = nc.sync if (rbi * nch + c) % 2 == 0 else nc.gpsimd
                    dma_eng.dma_start(
                        xt[:], x[rbi * P : (rbi + 1) * P, c * f : (c + 1) * f]
                    )
                    esc = epool.tile([P, f], f32, tag="esc")
                    nc.scalar.activation(
                        out=esc[:],
                        in_=xt[:],
                        func=mybir.ActivationFunctionType.Exp,
                        accum_out=part_all[:, rbi * nch + c : rbi * nch + c + 1],
                    )
            # batched epilogue for all row blocks at once
            s_all = spool.tile([P, rb_n], f32)
            nc.vector.tensor_reduce(
                s_all[:],
                part_all[:].rearrange("p (b c) -> p b c", c=nch),
                axis=mybir.AxisListType.X,
                op=mybir.AluOpType.add,
            )
            logz_all = spool.tile([P, rb_n], f32)
            nc.scalar.activation(
                out=logz_all[:], in_=s_all[:], func=mybir.ActivationFunctionType.Ln
            )
            ce_sb = spool.tile([P, rb_n], f32)
            nc.vector.tensor_tensor(
                out=ce_sb[:],
                in0=logz_all[:],
                in1=picked[:],
                op=mybir.AluOpType.subtract,
            )
            # p-major layout into ce_local (any permutation is fine: the
            # bottom-k mean is permutation invariant)
            nc.sync.dma_start(
                ce_local[:].rearrange("(p b) 1 -> p b", b=rb_n), ce_sb[:]
            )

            # all-gather the per-sample losses (tiny)
            nc.gpsimd.collective_compute(
                "AllGather",
                mybir.AluOpType.bypass,
                replica_groups=[list(range(n_cores))],
                ins=[ce_local[:].opt()],
                outs=[ce_all[:].opt()],
            )

            # replicate all ng losses into every partition (gpsimd ucode
            # cross-lane broadcast of partition 0), in two halves so the
            # DVE can start on the first half while the second broadcasts
            ce_row = selpool.tile([1, ng], f32)
            nc.sync.dma_start(ce_row[:], ce_all[:].rearrange("a 1 -> 1 a"))
            xrep = selpool.tile([P, ng], f32)
            nc.gpsimd.partition_broadcast(xrep[:, : ng // 2], ce_row[:, : ng // 2])
            nc.gpsimd.partition_broadcast(xrep[:, ng // 2 :], ce_row[:, ng // 2 :])

            # selection: split each whole-array pass between DVE (first half,
            # is_le / min) and ACT (second half, Sign / Relu with per-partition
            # bias) so the two engines run concurrently
            assert ng % 2 == 0 and ng == 2 * m
            h = ng // 2
            xa = xrep[:, :h]
            xb = xrep[:, h:]
            dummy = selpool.tile([P, 1], f32)
            ones = selpool.tile([P, P], f32)
            nc.vector.memset(ones[:], 1.0)

            def count_round(t_ap, tn_ap, name):
                # DVE: cA = #{x_A <= T}
                ca = selpool.tile([P, 1], f32, name=f"ca{name}")
                nc.vector.tensor_scalar(
                    out=dummy[:].broadcast_to([P, h]),
                    in0=xa,
                    scalar1=t_ap,
                    scalar2=None,
                    op0=mybir.AluOpType.is_le,
                    op1=mybir.AluOpType.add,
                    accum_out=ca[:],
                )
                # ACT: sgB = sum sign(x_B - T)  =>  cB = (h - sgB)/2 (ties ~0)
                scr = epool.tile([P, h], f32, tag="esc", name=f"scr{name}")
                sgb = selpool.tile([P, 1], f32, name=f"sgb{name}")
                nc.scalar.activation(
                    out=scr[:],
                    in_=xb,
                    func=mybir.ActivationFunctionType.Sign,
                    bias=tn_ap,
                    scale=1.0,
                    accum_out=sgb[:],
                )
                # u = cA - sgB/2 ; cnt >= m  <=>  u >= m - h/2
                u = selpool.tile([P, 1], f32, name=f"u{name}")
                nc.vector.tensor_scalar(
                    out=u[:], in0=sgb[:], scalar1=-0.5, scalar2=ca[:],
                    op0=mybir.AluOpType.mult, op1=mybir.AluOpType.add,
                )
                ge = selpool.tile([P, 1], f32, name=f"ge{name}")
                nc.vector.tensor_scalar(
                    out=ge[:], in0=u[:], scalar1=float(m) - h / 2.0, scalar2=None,
                    op0=mybir.AluOpType.is_ge,
                )
                # partition-sum of ge, replicated, via ones-matmul
                g = ppool.tile([P, 1], f32, name=f"g{name}", tag="gps")
                nc.tensor.matmul(out=g[:], lhsT=ones[:], rhs=ge[:], start=True, stop=True)
                return g

            # round 1: thresholds io1 = (p+1)*S1; biased lo1 = -g1*S1
            g1 = count_round(io1[:], io1n[:], "1")
            lo1 = selpool.tile([P, 1], f32)
            nc.vector.tensor_scalar(
                out=lo1[:], in0=g1[:], scalar1=-S1, scalar2=None,
                op0=mybir.AluOpType.mult,
            )
            # round 2: T2 = lo1 + (range1 + (p+1)*S2); negT2 = g1*S1 - io2
            t2 = selpool.tile([P, 1], f32)
            nc.vector.tensor_tensor(
                out=t2[:], in0=lo1[:], in1=io2[:], op=mybir.AluOpType.add
            )
            t2n = selpool.tile([P, 1], f32)
            nc.vector.tensor_scalar(
                out=t2n[:], in0=g1[:], scalar1=S1, scalar2=io2[:],
                op0=mybir.AluOpType.mult, op1=mybir.AluOpType.subtract,
            )
            g2 = count_round(t2[:], t2n[:], "2")
            lo2 = selpool.tile([P, 1], f32)
            nc.vector.tensor_scalar(
                out=lo2[:], in0=g2[:], scalar1=-S2, scalar2=lo1[:],
                op0=mybir.AluOpType.mult, op1=mybir.AluOpType.add,
            )
            # final threshold t = true_lo2 + S2 (un-bias by the two ranges).
            # Bracket width S2 ~ 2e-3: the min-sum correction keeps the
            # result error ~ (#ties within S2 of v_m) * S2 / m ~ 1e-6 abs.
            c_t = 128.0 * S1 + 128.0 * S2 + S2
            tf = selpool.tile([P, 1], f32)
            nc.vector.tensor_scalar(
                out=tf[:], in0=lo2[:], scalar1=c_t, scalar2=None,
                op0=mybir.AluOpType.add,
            )
            # bottom-m mean, tie-corrected.  With h = m the t terms cancel:
            #   res = (sum_A min(x,t) - sum_B relu(t-x)) / m
            n_sc = 4
            sc = h // n_sc
            smin_cols = selpool.tile([P, n_sc], f32)
            for k in range(n_sc):
                nc.vector.tensor_scalar(
                    out=dummy[:].broadcast_to([P, sc]),
                    in0=xa[:, k * sc : (k + 1) * sc],
                    scalar1=tf[:],
                    scalar2=None,
                    op0=mybir.AluOpType.min,
                    op1=mybir.AluOpType.add,
                    accum_out=smin_cols[:, k : k + 1],
                )
            relu_cols = selpool.tile([P, n_sc], f32)
            for k in range(n_sc):
                scr = epool.tile([P, sc], f32, tag="esc", name=f"scrr{k}")
                nc.scalar.activation(
                    out=scr[:],
                    in_=xb[:, k * sc : (k + 1) * sc],
                    func=mybir.ActivationFunctionType.Relu,
                    bias=tf[:],
                    scale=-1.0,
                    accum_out=relu_cols[:, k : k + 1],
                )
            smin = selpool.tile([P, 1], f32)
            nc.vector.reduce_sum(smin[:], smin_cols[:], axis=mybir.AxisListType.X)
            srelu = selpool.tile([P, 1], f32)
            nc.vector.reduce_sum(srelu[:], relu_cols[:], axis=mybir.AxisListType.X)
            d = selpool.tile([P, 1], f32)
            nc.vector.tensor_tensor(
                out=d[:], in0=smin[:], in1=srelu[:], op=mybir.AluOpType.subtract
            )
            res = selpool.tile([P, 1], f32)
            nc.vector.tensor_scalar(
                out=res[:], in0=d[:], scalar1=1.0 / m, scalar2=None,
                op0=mybir.AluOpType.mult,
            )
            nc.sync.dma_start(out[:], res[0:1, :])

    if not nc.is_finalized():
        nc.finalize()
    return nc


def make_host_inputs(x_full, labels_full, n_cores, r, v):
    """Shard rows across cores and build the per-core input maps."""
    rb_n = r // P
    io1 = ((np.arange(P, dtype=np.float64) + 1) * S1).astype(np.float32)
    io2 = (128 * S1 + (np.arange(P, dtype=np.float64) + 1) * S2).astype(np.float32)
    in_maps = []
    for c in range(n_cores):
        rows = slice(c * r, (c + 1) * r)
        xs = np.ascontiguousarray(x_full[rows], dtype=np.float32)
        lb = np.asarray(labels_full[rows], dtype=np.int64)
        offs_flat = (np.arange(r, dtype=np.int64) * v + lb).astype(np.int32)
        offs = np.ascontiguousarray(offs_flat.reshape(rb_n, P).T)
        in_maps.append(
            {
                "x": xs,
                "offs": offs,
                "iota1": io1.reshape(P, 1),
                "iota1n": (-io1).reshape(P, 1),
                "iota2": io2.reshape(P, 1),
            }
        )
    return in_maps


def run(inputs, trace=False, f=4000):
    from concourse.bass_utils import run_bass_kernel_spmd

    x_full = np.asarray(inputs["outputs"], dtype=np.float32)
    labels_full = np.asarray(inputs["labels"])
    n, v = x_full.shape
    r = n // N_CORES
    nc = build_nc(N_CORES, r, v, f)
    in_maps = make_host_inputs(x_full, labels_full, N_CORES, r, v)
    try:
        res = run_bass_kernel_spmd(
            nc, in_maps, list(range(N_CORES)), trace=trace
        )
    except Exception:
        # transient device errors (e.g. a wedged core from a prior run)
        # usually clear on retry
        res = run_bass_kernel_spmd(
            nc, in_maps, list(range(N_CORES)), trace=trace
        )
    val = np.asarray(res.results[0]["out"], dtype=np.float32).reshape(-1)[0]
    return np.asarray(val, dtype=np.float32), res


def kernel(outputs=None, labels=None, **_ignored):
    out, _ = run({"outputs": outputs, "labels": labels})
    return out


# revision 30
# speedup vs baseline: 1.2044x; 1.2044x over previous
"""Bottom-k cross-entropy loss on 8 Trainium2 NeuronCores.

Per-sample CE over [8192, 32000] logits, then mean of the 4096 smallest
losses.  Data-parallel: rows sharded across 8 cores; each core streams its
131MB shard once (memory-bound), computes local CE via one fused
exp+accumulate pass on the scalar engine, all-gathers the 8192 losses
(tiny), and every core redundantly runs an exact threshold-refinement
selection (2 rounds x 128 brackets, then a tie-corrected min/relu sum,
split across the vector and scalar engines) to produce the bottom-k mean.

Selection math: brackets are multiples of 2^-9 < 32, so all threshold
arithmetic is exact in f32.  With t >= v_(m) within one final bracket,
  mean_bottom_m = (sum_A min(x, t) - sum_B relu(t - x)) / m
(the t terms cancel exactly when m = N/2) is exact up to
(C(t)-m)*bracket_width/m ~ 1e-6 absolute.
"""

import numpy as np

N_CORES = 8
N_FULL, V_FULL = 8192, 32000
P = 128

# bracket steps: ranges 32, 0.25, 2^-9; all CE values lie in (0, 32]
S1, S2, S3 = 2.0**-2, 2.0**-9, 2.0**-16


def build_nc(n_cores, r, v, f):
    """Build the SPMD Bass program (identical on every core)."""
    from concourse import bass, bacc, mybir, tile

    assert r % P == 0 and v % f == 0
    rb_n = r // P
    nch = v // f
    ng = r * n_cores
    m = ng // 2
    f32 = mybir.dt.float32

    nc = bacc.Bacc()
    x = nc.declare_dram_parameter("x", [r, v], f32, isOutput=False)
    offs = nc.declare_dram_parameter("offs", [P, rb_n], mybir.dt.int32, isOutput=False)
    iota1 = nc.declare_dram_parameter("iota1", [P, 1], f32, isOutput=False)
    iota1n = nc.declare_dram_parameter("iota1n", [P, 1], f32, isOutput=False)
    iota2 = nc.declare_dram_parameter("iota2", [P, 1], f32, isOutput=False)
    out = nc.declare_dram_parameter("out", [1, 1], f32, isOutput=True)

    with tile.TileContext(nc) as tc:
        with (
            tc.tile_pool(name="dram", bufs=1, space="DRAM") as dpool,
            tc.tile_pool(name="consts", bufs=1) as cpool,
            tc.tile_pool(name="xs", bufs=5) as xpool,
            tc.tile_pool(name="es", bufs=2) as epool,
            tc.tile_pool(name="stats", bufs=2) as spool,
            tc.tile_pool(name="sel", bufs=1) as selpool,
            tc.tile_pool(name="psum", bufs=2, space="PSUM") as ppool,
        ):
            ce_local = dpool.tile([r, 1], f32, name="ce_local")
            ce_all = dpool.tile([ng, 1], f32, addr_space="Shared", name="ce_all")
            offs_sb = cpool.tile([P, rb_n], mybir.dt.int32)
            nc.gpsimd.dma_start(offs_sb[:], offs[:])
            io1 = cpool.tile([P, 1], f32)
            nc.gpsimd.dma_start(io1[:], iota1[:])
            io1n = cpool.tile([P, 1], f32)
            nc.gpsimd.dma_start(io1n[:], iota1n[:])
            io2 = cpool.tile([P, 1], f32)
            nc.gpsimd.dma_start(io2[:], iota2[:])

            # tiny dummy partition_broadcast: forces the gpsimd ucode library
            # load to happen here (gpsimd is idle during streaming) instead of
            # right before the real broadcast in the latency-critical tail
            dsrc = cpool.tile([1, 4], f32)
            nc.vector.memset(dsrc[:], 0.0)
            dout = cpool.tile([P, 4], f32)
            nc.gpsimd.partition_broadcast(dout[:], dsrc[:])

            # gather picked logits: x.flat[row*v + label] for each local row
            picked = cpool.tile([P, rb_n], f32)
            x_flat = x[:].rearrange("a b -> (a b) ()")
            for rbi in range(rb_n):
                nc.gpsimd.indirect_dma_start(
                    out=picked[:, rbi : rbi + 1],
                    out_offset=None,
                    in_=x_flat,
                    in_offset=bass.IndirectOffsetOnAxis(
                        ap=offs_sb[:, rbi : rbi + 1], axis=0
                    ),
                )

            # streaming pass: pure DMA + fused exp/accumulate, no mid-stream
            # epilogues (table switches and dependent stores would stall the
            # sync HWDGE ring and the ACT queue at row-block boundaries)
            part_all = spool.tile([P, rb_n * nch], f32)
            for rbi in range(rb_n):
                for c in range(nch):
                    xt = xpool.tile([P, f], f32, tag="xt")
                    nc.sync.dma_start(
                        xt[:], x[rbi * P : (rbi + 1) * P, c * f : (c + 1) * f]
                    )
                    esc = epool.tile([P, f], f32, tag="esc")
                    nc.scalar.activation(
                        out=esc[:],
                        in_=xt[:],
                        func=mybir.ActivationFunctionType.Exp,
                        accum_out=part_all[:, rbi * nch + c : rbi * nch + c + 1],
                    )
            # batched epilogue for all row blocks at once
            s_all = spool.tile([P, rb_n], f32)
            nc.vector.tensor_reduce(
                s_all[:],
                part_all[:].rearrange("p (b c) -> p b c", c=nch),
                axis=mybir.AxisListType.X,
                op=mybir.AluOpType.add,
            )
            logz_all = spool.tile([P, rb_n], f32)
            nc.scalar.activation(
                out=logz_all[:], in_=s_all[:], func=mybir.ActivationFunctionType.Ln
            )
            ce_sb = spool.tile([P, rb_n], f32)
            nc.vector.tensor_tensor(
                out=ce_sb[:],
                in0=logz_all[:],
                in1=picked[:],
                op=mybir.AluOpType.subtract,
            )
            # p-major layout into ce_local (any permutation is fine: the
            # bottom-k mean is permutation invariant)
            nc.sync.dma_start(
                ce_local[:].rearrange("(p b) 1 -> p b", b=rb_n), ce_sb[:]
            )

            # all-gather the per-sample losses (tiny)
            nc.gpsimd.collective_compute(
                "AllGather",
                mybir.AluOpType.bypass,
                replica_groups=[list(range(n_cores))],
                ins=[ce_local[:].opt()],
                outs=[ce_all[:].opt()],
            )

            # replicate all ng losses into every partition (gpsimd ucode
            # cross-lane broadcast of partition 0), in two halves so the
            # DVE can start on the first half while the second broadcasts
            ce_row = selpool.tile([1, ng], f32)
            nc.sync.dma_start(ce_row[:], ce_all[:].rearrange("a 1 -> 1 a"))
            xrep = selpool.tile([P, ng], f32)
            nc.gpsimd.partition_broadcast(xrep[:, : ng // 2], ce_row[:, : ng // 2])
            nc.gpsimd.partition_broadcast(xrep[:, ng // 2 :], ce_row[:, ng // 2 :])

            # selection: split each whole-array pass between DVE (first half,
            # is_le / min) and ACT (second half, Sign / Relu with per-partition
            # bias) so the two engines run concurrently
            assert ng % 2 == 0 and ng == 2 * m
            h = ng // 2
            xa = xrep[:, :h]
            xb = xrep[:, h:]
            dummy = selpool.tile([P, 1], f32)
            ones = selpool.tile([P, P], f32)
            nc.vector.memset(ones[:], 1.0)

            def count_round(t_ap, tn_ap, name):
                # DVE: cA = #{x_A <= T}
                ca = selpool.tile([P, 1], f32, name=f"ca{name}")
                nc.vector.tensor_scalar(
                    out=dummy[:].broadcast_to([P, h]),
                    in0=xa,
                    scalar1=t_ap,
                    scalar2=None,
                    op0=mybir.AluOpType.is_le,
                    op1=mybir.AluOpType.add,
                    accum_out=ca[:],
                )
                # ACT: sgB = sum sign(x_B - T)  =>  cB = (h - sgB)/2 (ties ~0)
                scr = epool.tile([P, h], f32, tag="esc", name=f"scr{name}")
                sgb = selpool.tile([P, 1], f32, name=f"sgb{name}")
                nc.scalar.activation(
                    out=scr[:],
                    in_=xb,
                    func=mybir.ActivationFunctionType.Sign,
                    bias=tn_ap,
                    scale=1.0,
                    accum_out=sgb[:],
                )
                # u = cA - sgB/2 ; cnt >= m  <=>  u >= m - h/2
                u = selpool.tile([P, 1], f32, name=f"u{name}")
                nc.vector.tensor_scalar(
                    out=u[:], in0=sgb[:], scalar1=-0.5, scalar2=ca[:],
                    op0=mybir.AluOpType.mult, op1=mybir.AluOpType.add,
                )
                ge = selpool.tile([P, 1], f32, name=f"ge{name}")
                nc.vector.tensor_scalar(
                    out=ge[:], in0=u[:], scalar1=float(m) - h / 2.0, scalar2=None,
                    op0=mybir.AluOpType.is_ge,
                )
                # partition-sum of ge, replicated, via ones-matmul
                g = ppool.tile([P, 1], f32, name=f"g{name}", tag="gps")
                nc.tensor.matmul(out=g[:], lhsT=ones[:], rhs=ge[:], start=True, stop=True)
                return g

            # round 1: thresholds io1 = (p+1)*S1; biased lo1 = -g1*S1
            g1 = count_round(io1[:], io1n[:], "1")
            lo1 = selpool.tile([P, 1], f32)
            nc.vector.tensor_scalar(
                out=lo1[:], in0=g1[:], scalar1=-S1, scalar2=None,
                op0=mybir.AluOpType.mult,
            )
            # round 2: T2 = lo1 + (range1 + (p+1)*S2); negT2 = g1*S1 - io2
            t2 = selpool.tile([P, 1], f32)
            nc.vector.tensor_tensor(
                out=t2[:], in0=lo1[:], in1=io2[:], op=mybir.AluOpType.add
            )
            t2n = selpool.tile([P, 1], f32)
            nc.vector.tensor_scalar(
                out=t2n[:], in0=g1[:], scalar1=S1, scalar2=io2[:],
                op0=mybir.AluOpType.mult, op1=mybir.AluOpType.subtract,
            )
            g2 = count_round(t2[:], t2n[:], "2")
            lo2 = selpool.tile([P, 1], f32)
            nc.vector.tensor_scalar(
                out=lo2[:], in0=g2[:], scalar1=-S2, scalar2=lo1[:],
                op0=mybir.AluOpType.mult, op1=mybir.AluOpType.add,
            )
            # final threshold t = true_lo2 + S2 (un-bias by the two ranges).
            # Bracket width S2 ~ 2e-3: the min-sum correction keeps the
            # result error ~ (#ties within S2 of v_m) * S2 / m ~ 1e-6 abs.
            c_t = 128.0 * S1 + 128.0 * S2 + S2
            tf = selpool.tile([P, 1], f32)
            nc.vector.tensor_scalar(
                out=tf[:], in0=lo2[:], scalar1=c_t, scalar2=None,
                op0=mybir.AluOpType.add,
            )
            # bottom-m mean, tie-corrected.  With h = m the t terms cancel:
            #   res = (sum_A min(x,t) - sum_B relu(t-x)) / m
            n_sc = 4
            sc = h // n_sc
            smin_cols = selpool.tile([P, n_sc], f32)
            for k in range(n_sc):
                nc.vector.tensor_scalar(
                    out=dummy[:].broadcast_to([P, sc]),
                    in0=xa[:, k * sc : (k + 1) * sc],
                    scalar1=tf[:],
                    scalar2=None,
                    op0=mybir.AluOpType.min,
                    op1=mybir.AluOpType.add,
                    accum_out=smin_cols[:, k : k + 1],
                )
            relu_cols = selpool.tile([P, n_sc], f32)
            for k in range(n_sc):
                scr = epool.tile([P, sc], f32, tag="esc", name=f"scrr{k}")
                nc.scalar.activation(
                    out=scr[:],
                    in_=xb[:, k * sc : (k + 1) * sc],
                    func=mybir.ActivationFunctionType.Relu,
                    bias=tf[:],
                    scale=-1.0,
                    accum_out=relu_cols[:, k : k + 1],
                )
            smin = selpool.tile([P, 1], f32)
            nc.vector.reduce_sum(smin[:], smin_cols[:], axis=mybir.AxisListType.X)
            srelu = selpool.tile([P, 1], f32)
            nc.vector.reduce_sum(srelu[:], relu_cols[:], axis=mybir.AxisListType.X)
            d = selpool.tile([P, 1], f32)
            nc.vector.tensor_tensor(
                out=d[:], in0=smin[:], in1=srelu[:], op=mybir.AluOpType.subtract
            )
            res = selpool.tile([P, 1], f32)
            nc.vector.tensor_scalar(
                out=res[:], in0=d[:], scalar1=1.0 / m, scalar2=None,
                op0=mybir.AluOpType.mult,
            )
            nc.sync.dma_start(out[:], res[0:1, :])

    if not nc.is_finalized():
        nc.finalize()
    return nc


def make_host_inputs(x_full, labels_full, n_cores, r, v):
    """Shard rows across cores and build the per-core input maps."""
    rb_n = r // P
    io1 = ((np.arange(P, dtype=np.float64) + 1) * S1).astype(np.float32)
    io2 = (128 * S1 + (np.arange(P, dtype=np.float64) + 1) * S2).astype(np.float32)
    in_maps = []
    for c in range(n_cores):
        rows = slice(c * r, (c + 1) * r)
        xs = np.ascontiguousarray(x_full[rows], dtype=np.float32)
        lb = np.asarray(labels_full[rows], dtype=np.int64)
        offs_flat = (np.arange(r, dtype=np.int64) * v + lb).astype(np.int32)
        offs = np.ascontiguousarray(offs_flat.reshape(rb_n, P).T)
        in_maps.append(
            {
                "x": xs,
                "offs": offs,
                "iota1": io1.reshape(P, 1),
                "iota1n": (-io1).reshape(P, 1),
                "iota2": io2.reshape(P, 1),
            }
        )
    return in_maps


def run(inputs, trace=False, f=4000):
    from concourse.bass_utils import run_bass_kernel_spmd

    x_full = np.asarray(inputs["outputs"], dtype=np.float32)
    labels_full = np.asarray(inputs["labels"])
    n, v = x_full.shape
    r = n // N_CORES
    nc = build_nc(N_CORES, r, v, f)
    in_maps = make_host_inputs(x_full, labels_full, N_CORES, r, v)
    try:
        res = run_bass_kernel_spmd(
            nc, in_maps, list(range(N_CORES)), trace=trace
        )
    except Exception:
        # transient device errors (e.g. a wedged core from a prior run)
        # usually clear on retry
        res = run_bass_kernel_spmd(
            nc, in_maps, list(range(N_CORES)), trace=trace
        )
    val = np.asarray(res.results[0]["out"], dtype=np.float32).reshape(-1)[0]
    return np.asarray(val, dtype=np.float32), res


def kernel(outputs=None, labels=None, **_ignored):
    out, _ = run({"outputs": outputs, "labels": labels})
    return out
